# revision 7
# baseline (speedup 1.0000x reference)
"""DGLJTMPN message-passing network on 8 Trainium2 NeuronCores (Bass/Tile).

Algorithm (mathematically identical to the reference):
  The loopy-BP line-graph aggregation  accum = segment_sum(msg[lg_src], lg_dst)
  is rewritten with node-level sums:  accum[e] = S[src[e]] - (backtracking
  partners), where S = segment_sum(msg, edge_dst).  The missing/backtracking
  pairs (the complement of the given lg list w.r.t. the full line graph) are
  folded into extra host-built one-hot "virtual columns", so each edge reads
  exactly one row of U = (S + node_alpha) @ W_h per iteration.

Sharding: nodes/edges/graphs are split into 8 contiguous graph-aligned
ranges; edges live on the core owning their dst node, so S/U shards are
disjoint.  Per iteration each core computes its U shard, an AllGather
replicates U, and a runtime-indexed dma_gather fetches U[src[e]] (two int16
half-table passes).  Scatter-adds are one-hot matmuls on the tensor engine.
"""

import numpy as np
import ml_dtypes

P = 128
SC = 4            # edge chunks per node tile
EDGE_CAP = SC * P
H = 256
GROUP = 1024      # edges per dma_gather call (8 chunks of 128; SWDGE ring caps a single gather near 2048 descriptors)
GPC = GROUP // P  # chunks per group = 16
N_CORES = 8
DEPTH = 4
GCAP = 2 * P      # max graphs per core

F32 = np.float32
BF16 = ml_dtypes.bfloat16


# ======================================================================
# Host preprocessing
# ======================================================================

def _full_line_graph_keys(src, dst, E, N):
    indeg = np.bincount(dst, minlength=N)
    idx_sorted = np.argsort(dst, kind="stable")
    ptr = np.concatenate([[0], np.cumsum(indeg)]).astype(np.int64)
    counts = indeg[src]
    total = int(counts.sum())
    lg_dst = np.repeat(np.arange(E, dtype=np.int64), counts)
    cum = np.cumsum(counts) - counts
    within = np.arange(total) - np.repeat(cum, counts)
    lg_src = idx_sorted[np.repeat(ptr[src], counts) + within]
    return lg_src * E + lg_dst


def _prep(inputs, n_cores=N_CORES):
    x_nodes = np.asarray(inputs["x_nodes"], F32)
    x_edges = np.asarray(inputs["x_edges"], F32)
    tree_m = np.asarray(inputs["tree_m"], F32)
    W_i = np.asarray(inputs["W_i"], F32)
    W_h = np.asarray(inputs["W_h"], F32)
    W_o = np.asarray(inputs["W_o"], F32)
    b_o = np.asarray(inputs["b_o"], F32)
    src = np.asarray(inputs["edge_src"], np.int64)
    dst = np.asarray(inputs["edge_dst"], np.int64)
    lg_src = np.asarray(inputs["lg_src"], np.int64)
    lg_dst = np.asarray(inputs["lg_dst"], np.int64)
    tgt_nodes = np.asarray(inputs["tgt_nodes"], np.int64)
    tree_eid = np.asarray(inputs["tree_eid"], np.int64)
    graph_ids = np.asarray(inputs["graph_ids"], np.int64)
    n_graphs = int(inputs["n_graphs"])

    N = x_nodes.shape[0]
    E = x_edges.shape[0]
    AF = x_nodes.shape[1]
    KF = AF + x_edges.shape[1]

    # corrections: full-line-graph pairs missing from the given lg list
    full_keys = _full_line_graph_keys(src, dst, E, N)
    given_keys = lg_src * E + lg_dst
    missing = np.setdiff1d(full_keys, given_keys)
    assert np.setdiff1d(given_keys, full_keys).size == 0
    miss_e1 = (missing // E).astype(np.int64)
    miss_e2 = (missing % E).astype(np.int64)
    assert np.all(dst[miss_e1] == src[miss_e2])
    order = np.argsort(miss_e2, kind="stable")
    miss_e1, miss_e2 = miss_e1[order], miss_e2[order]
    corr_e2, corr_start = np.unique(miss_e2, return_index=True)
    corr_partners = {}
    for i, e2 in enumerate(corr_e2):
        lo = corr_start[i]
        hi = corr_start[i + 1] if i + 1 < len(corr_e2) else len(miss_e2)
        corr_partners[int(e2)] = miss_e1[lo:hi]
    virt_nodes = src[corr_e2] if len(corr_e2) else np.array([], np.int64)
    vdemand = np.bincount(virt_nodes, minlength=N)
    corr_by_node = {}
    for e2 in corr_e2:
        corr_by_node.setdefault(int(src[e2]), []).append(int(e2))

    # graph-aligned node cuts
    gcnt = np.bincount(graph_ids, minlength=n_graphs)
    gcum = np.concatenate([[0], np.cumsum(gcnt)])
    cuts = [0]
    for c in range(1, n_cores):
        g = int(np.argmin(np.abs(gcum - c * N / n_cores)))
        cuts.append(int(gcum[g]))
    cuts.append(N)
    cuts = np.asarray(cuts, np.int64)
    assert np.all(np.diff(cuts) > 0)

    indeg = np.bincount(dst, minlength=N)
    assert indeg.max() <= EDGE_CAP
    edges_by_dst = np.argsort(dst, kind="stable")
    eptr = np.concatenate([[0], np.cumsum(indeg)]).astype(np.int64)
    tdeg = np.bincount(tgt_nodes, minlength=N)
    tpairs_by_tgt = np.argsort(tgt_nodes, kind="stable")
    tptr = np.concatenate([[0], np.cumsum(tdeg)]).astype(np.int64)

    # tile packing
    per_core_tiles = []
    for c in range(n_cores):
        nlo, nhi = int(cuts[c]), int(cuts[c + 1])
        tiles, cur, cur_slots, cur_edges = [], [], 0, 0
        for n in range(nlo, nhi):
            ns, ne = 1 + int(vdemand[n]), int(indeg[n])
            cap = P - 1 if (len(tiles) == 0 and c in (0, n_cores // 2)) else P
            if cur and (cur_slots + ns > cap or cur_edges + ne > EDGE_CAP):
                tiles.append(cur)
                cur, cur_slots, cur_edges = [], 0, 0
            cur.append(n)
            cur_slots += ns
            cur_edges += ne
        if cur:
            tiles.append(cur)
        per_core_tiles.append(tiles)

    Kn = max(len(t) for t in per_core_tiles)
    Kn = -(-Kn // 4) * 4
    CORE_ROWS = P * Kn
    R = CORE_ROWS * n_cores
    HALF = R // 2
    assert HALF <= 32767
    E_slab = Kn * EDGE_CAP
    n_groups = E_slab // GROUP

    SCT = 1
    for c in range(n_cores):
        for tile in per_core_tiles[c]:
            SCT = max(SCT, -(-int(sum(tdeg[n] for n in tile)) // P))
    T_slab = Kn * SCT * P

    meta = dict(N=N, E=E, AF=AF, KF=KF, Kn=Kn, SCT=SCT, CORE_ROWS=CORE_ROWS,
                R=R, HALF=HALF, E_slab=E_slab, n_groups=n_groups,
                T_slab=T_slab, n_cores=n_cores, n_graphs=n_graphs,
                n_corr=len(corr_e2))

    z0_row = P - 1
    z1_row = (n_cores // 2) * CORE_ROWS + (P - 1)

    # node slot assignment (global)
    node_row = np.full(N, -1, np.int64)
    virt_slot = {}
    for c in range(n_cores):
        for t, tile in enumerate(per_core_tiles[c]):
            j = 0
            for n in tile:
                node_row[n] = CORE_ROWS * c + P * t + j
                j += 1
                for e2 in corr_by_node.get(n, []):
                    virt_slot[e2] = CORE_ROWS * c + P * t + j
                    j += 1
            assert j <= P
    assert np.all(node_row >= 0)
    edge_row = node_row[src].copy()
    for e2, row in virt_slot.items():
        edge_row[e2] = row

    glo_ghi = []
    per_core = []
    for c in range(n_cores):
        nlo, nhi = int(cuts[c]), int(cuts[c + 1])
        tiles = per_core_tiles[c]
        glo = int(graph_ids[nlo])
        ghi = int(graph_ids[nhi - 1]) + 1
        assert ghi - glo <= GCAP
        glo_ghi.append((glo, ghi))

        sel = np.zeros((Kn, SC, P, P), F32)
        seltree = np.zeros((Kn, SCT, P, P), F32)
        tree_slab = np.zeros((T_slab, H), F32)
        xe_catT = np.zeros((KF, E_slab), F32)
        xnodesT = np.zeros((AF + 1, CORE_ROWS), F32)
        xnodesT[AF, :] = 1.0
        poolw = np.zeros((Kn, P, GCAP), F32)
        idx_rows = np.full(E_slab, -1, np.int64)

        for t, tile in enumerate(tiles):
            base = CORE_ROWS * c + P * t
            pos_of_edge = {}
            k = 0
            for n in tile:
                j = int(node_row[n] - base)
                xnodesT[:AF, P * t + j] = x_nodes[n]
                g = int(graph_ids[n])
                poolw[t, j, g - glo] = 1.0 / max(int(gcnt[g]), 1)
                for e in edges_by_dst[eptr[n]:eptr[n + 1]]:
                    slab_pos = EDGE_CAP * t + k
                    pos_of_edge[int(e)] = k
                    sel[t, k // P, k % P, j] = 1.0
                    idx_rows[slab_pos] = edge_row[e]
                    xe_catT[:AF, slab_pos] = x_nodes[src[e]]
                    xe_catT[AF:, slab_pos] = x_edges[e]
                    k += 1
            assert k <= EDGE_CAP
            # virtual columns
            for n in tile:
                for vi, e2 in enumerate(corr_by_node.get(n, [])):
                    jv = int(virt_slot[e2] - base)
                    partners = set(corr_partners[e2].tolist())
                    for e in edges_by_dst[eptr[n]:eptr[n + 1]]:
                        if int(e) in partners:
                            continue
                        kk = pos_of_edge[int(e)]
                        sel[t, kk // P, kk % P, jv] = 1.0
            # tree pairs
            kt = 0
            for n in tile:
                j = int(node_row[n] - base)
                nvirt = len(corr_by_node.get(n, []))
                for pidx in tpairs_by_tgt[tptr[n]:tptr[n + 1]]:
                    tree_slab[SCT * P * t + kt] = tree_m[tree_eid[pidx]]
                    seltree[t, kt // P, kt % P, j] = 1.0
                    for vi in range(nvirt):
                        seltree[t, kt // P, kt % P, j + 1 + vi] = 1.0
                    kt += 1
            assert kt <= SCT * P

        in0 = (idx_rows >= 0) & (idx_rows < HALF)
        in1 = idx_rows >= HALF
        idx0 = np.where(in0, idx_rows, z0_row)
        idx1 = np.where(in1, idx_rows - HALF, z1_row - HALF)
        assert 0 <= idx0.min() and idx0.max() < HALF
        assert 0 <= idx1.min() and idx1.max() < HALF

        def wrap(idx):
            # -> [128, n_groups, GROUP//16] with j = col*16 + (p % 16)
            w = idx.reshape(n_groups, GROUP // 16, 16)   # [g, col, p16]
            w = np.transpose(w, (2, 0, 1))               # [p16, g, col]
            w = np.tile(w, (P // 16, 1, 1))
            return np.ascontiguousarray(w.astype(np.int16))

        # sel regrouped for gather-group-major DMA:
        # [n_groups, 128(e), GPC, 128(j)]
        selg = np.transpose(
            sel.reshape(n_groups, GPC, P, P), (0, 2, 1, 3))
        per_core.append(dict(
            xe_catT=xe_catT,
            sel=np.ascontiguousarray(selg.astype(BF16)),
            seltree=seltree,
            tree_slab=tree_slab,
            xnodesT=xnodesT,
            poolw=poolw,
            idx0=wrap(idx0),
            idx1=wrap(idx1),
            wi=W_i.copy(),
            wh=W_h.copy(),
            wo1=np.ascontiguousarray(
                np.concatenate([W_o[:AF], b_o[None, :]], 0)),
            wo2=np.ascontiguousarray(W_o[AF:]),
        ))

    return per_core, meta, glo_ghi


# ======================================================================
# Bass program
# ======================================================================

def _build(meta):
    import concourse.bacc as bacc
    import concourse.tile as tile
    from concourse import mybir

    Kn, SCT = meta["Kn"], meta["SCT"]
    CORE_ROWS, R, HALF = meta["CORE_ROWS"], meta["R"], meta["HALF"]
    E_slab, n_groups, T_slab = meta["E_slab"], meta["n_groups"], meta["T_slab"]
    KF, AF = meta["KF"], meta["AF"]
    TPG = GPC // SC            # node tiles per gather group (4)

    f32, bf16, i16 = mybir.dt.float32, mybir.dt.bfloat16, mybir.dt.int16
    RELU = mybir.ActivationFunctionType.Relu
    ADD = mybir.AluOpType.add

    nc = bacc.Bacc("TRN2", target_bir_lowering=False, num_devices=N_CORES)

    # kernel I/O
    xe_in = nc.dram_tensor("xe_catT", [KF, E_slab], f32, kind="ExternalInput")
    sel_in = nc.dram_tensor("sel", [n_groups, P, GPC, P], bf16,
                            kind="ExternalInput")
    seltree_in = nc.dram_tensor("seltree", [Kn, SCT, P, P], f32,
                                kind="ExternalInput")
    tree_in = nc.dram_tensor("tree_slab", [T_slab, H], f32,
                             kind="ExternalInput")
    xn_in = nc.dram_tensor("xnodesT", [AF + 1, CORE_ROWS], f32,
                           kind="ExternalInput")
    poolw_in = nc.dram_tensor("poolw", [Kn, P, GCAP], f32,
                              kind="ExternalInput")
    idx0_in = nc.dram_tensor("idx0", [P, n_groups, GROUP // 16], i16,
                             kind="ExternalInput")
    idx1_in = nc.dram_tensor("idx1", [P, n_groups, GROUP // 16], i16,
                             kind="ExternalInput")
    wi_in = nc.dram_tensor("wi", [KF, H], f32, kind="ExternalInput")
    wh_in = nc.dram_tensor("wh", [H, H], f32, kind="ExternalInput")
    wo1_in = nc.dram_tensor("wo1", [AF + 1, H], f32, kind="ExternalInput")
    wo2_in = nc.dram_tensor("wo2", [H, H], f32, kind="ExternalInput")
    out_t = nc.dram_tensor("out", [GCAP, H], f32, kind="ExternalOutput")

    # internal DRAM
    msgin_hbm = nc.dram_tensor("msgin_hbm", [n_groups, P, GPC, H], f32)
    naT_hbm = nc.dram_tensor("naT_hbm", [Kn, P, 2, P], f32)
    ag_in = nc.dram_tensor("ag_in", [CORE_ROWS, H], f32)
    U_tab = [
        nc.dram_tensor("U_A", [R, H], f32, addr_space="Shared"),
        nc.dram_tensor("U_B", [R, H], f32, addr_space="Shared"),
    ]

    with tile.TileContext(nc) as tc:
        with tc.tile_pool(name="const", bufs=1) as cpool, \
             tc.tile_pool(name="work", bufs=2) as wp, \
             tc.tile_pool(name="small", bufs=3) as sp, \
             tc.tile_pool(name="psum", bufs=2, space="PSUM") as pp, \
             tc.tile_pool(name="psum_acc", bufs=1, space="PSUM") as pacc:

            # ---- resident constants ----
            wi_t = cpool.tile([KF, H], f32)
            nc.sync.dma_start(out=wi_t[:], in_=wi_in[:])
            wh0 = cpool.tile([P, H], f32)
            wh1 = cpool.tile([P, H], f32)
            nc.sync.dma_start(out=wh0[:], in_=wh_in[0:P, :])
            nc.sync.dma_start(out=wh1[:], in_=wh_in[P:H, :])
            wo1_t = cpool.tile([AF + 1, H], f32)
            nc.sync.dma_start(out=wo1_t[:], in_=wo1_in[:])
            wo2_0 = cpool.tile([P, H], f32)
            wo2_1 = cpool.tile([P, H], f32)
            nc.sync.dma_start(out=wo2_0[:], in_=wo2_in[0:P, :])
            nc.sync.dma_start(out=wo2_1[:], in_=wo2_in[P:H, :])
            idx0_t = cpool.tile([P, n_groups, GROUP // 16], i16)
            idx1_t = cpool.tile([P, n_groups, GROUP // 16], i16)
            nc.sync.dma_start(out=idx0_t[:], in_=idx0_in[:])
            nc.sync.dma_start(out=idx1_t[:], in_=idx1_in[:])

            # ---- stage A: node_alpha^T ----
            for t in range(Kn):
                ps_na = [pp.tile([P, P], f32, tag=f"ps_s{i}", name=f"ps_na{i}")
                         for i in range(2)]
                for ct in range(SCT):
                    tr = sp.tile([P, H], f32, tag="tr")
                    nc.sync.dma_start(
                        out=tr[:],
                        in_=tree_in[(t * SCT + ct) * P:(t * SCT + ct + 1) * P, :])
                    st = sp.tile([P, P], f32, tag="st")
                    nc.sync.dma_start(out=st[:], in_=seltree_in[t, ct])
                    for s in range(2):
                        nc.tensor.matmul(
                            out=ps_na[s][:],
                            lhsT=tr[:, s * P:(s + 1) * P], rhs=st[:],
                            start=(ct == 0), stop=(ct == SCT - 1))
                na_sb = sp.tile([P, 2, P], f32, tag="na_sb")
                for s in range(2):
                    nc.vector.tensor_copy(out=na_sb[:, s, :], in_=ps_na[s][:])
                nc.sync.dma_start(out=naT_hbm[t], in_=na_sb[:])

            # ---- sweeps ----
            for sw in range(DEPTH):
                last = sw == DEPTH - 1
                if last:
                    psG = [pacc.tile([P, H], f32, tag=f"psG{i}", name=f"psG{i}")
                           for i in range(2)]
                for grp in range(n_groups):
                    # message pre-activation for this group
                    if sw == 0:
                        mi = wp.tile([P, GPC, H], f32, tag="mi")
                        xe_g = wp.tile([KF, GROUP], f32, tag="xe")
                        nc.sync.dma_start(
                            out=xe_g[:],
                            in_=xe_in[:, grp * GROUP:(grp + 1) * GROUP])
                        for k in range(GPC):
                            ps_mi = pp.tile([P, H], f32, tag="ps_big", name="ps_mi")
                            nc.tensor.matmul(
                                out=ps_mi[:],
                                lhsT=xe_g[:, k * P:(k + 1) * P],
                                rhs=wi_t[:], start=True, stop=True)
                            nc.vector.tensor_copy(out=mi[:, k, :], in_=ps_mi[:])
                        nc.sync.dma_start(out=msgin_hbm[grp], in_=mi[:])
                        pre = mi
                    else:
                        mi = wp.tile([P, GPC, H], f32, tag="mi")
                        nc.sync.dma_start(out=mi[:], in_=msgin_hbm[grp])
                        g0 = wp.tile([P, GPC, H], f32, tag="g0")
                        g1 = wp.tile([P, GPC, H], f32, tag="g1")
                        Up = U_tab[(sw + 1) % 2]
                        nc.gpsimd.dma_gather(
                            out_ap=g0[:], in_ap=Up[0:HALF, :],
                            idxs_ap=idx0_t[:, grp, :],
                            num_idxs=GROUP, num_idxs_reg=GROUP,
                            elem_size=H, queue_num=0)
                        nc.gpsimd.dma_gather(
                            out_ap=g1[:], in_ap=Up[HALF:R, :],
                            idxs_ap=idx1_t[:, grp, :],
                            num_idxs=GROUP, num_idxs_reg=GROUP,
                            elem_size=H, queue_num=0)
                        f0 = mi[:].rearrange("p a b -> p (a b)")
                        nc.vector.tensor_tensor(
                            out=f0, in0=f0,
                            in1=g0[:].rearrange("p a b -> p (a b)"), op=ADD)
                        nc.vector.tensor_tensor(
                            out=f0, in0=f0,
                            in1=g1[:].rearrange("p a b -> p (a b)"), op=ADD)
                        pre = mi
                    msg = wp.tile([P, GPC, H], bf16, tag="msg")
                    nc.scalar.activation(
                        out=msg[:].rearrange("p a b -> p (a b)"),
                        in_=pre[:].rearrange("p a b -> p (a b)"), func=RELU)
                    selg = wp.tile([P, GPC, P], bf16, tag="selg")
                    nc.sync.dma_start(out=selg[:], in_=sel_in[grp])
                    for tt in range(TPG):
                        t = grp * TPG + tt
                        psS = [pp.tile([P, P], f32, tag=f"ps_s{i}", name=f"psS{i}")
                               for i in range(2)]
                        for cc in range(SC):
                            k = tt * SC + cc
                            for s in range(2):
                                nc.tensor.matmul(
                                    out=psS[s][:],
                                    lhsT=msg[:, k, s * P:(s + 1) * P],
                                    rhs=selg[:, k, :],
                                    start=(cc == 0), stop=(cc == SC - 1))
                        na_t = sp.tile([P, 2, P], f32, tag="na_t")
                        nc.sync.dma_start(out=na_t[:], in_=naT_hbm[t])
                        TT = [sp.tile([P, P], f32, tag=f"TT{i}", name=f"TT{i}")
                              for i in range(2)]
                        for s in range(2):
                            nc.vector.tensor_tensor(
                                out=TT[s][:], in0=psS[s][:],
                                in1=na_t[:, s, :], op=ADD)
                        if not last:
                            psU = pp.tile([P, H], f32, tag="ps_big", name="psU")
                            nc.tensor.matmul(out=psU[:], lhsT=TT[0][:],
                                             rhs=wh0[:], start=True, stop=False)
                            nc.tensor.matmul(out=psU[:], lhsT=TT[1][:],
                                             rhs=wh1[:], start=False, stop=True)
                            u_sb = sp.tile([P, H], f32, tag="u_sb")
                            nc.vector.tensor_copy(out=u_sb[:], in_=psU[:])
                            nc.sync.dma_start(
                                out=ag_in[t * P:(t + 1) * P, :], in_=u_sb[:])
                        else:
                            xn_t = sp.tile([AF + 1, P], f32, tag="xn_t")
                            nc.sync.dma_start(
                                out=xn_t[:],
                                in_=xn_in[:, t * P:(t + 1) * P])
                            psH = pp.tile([P, H], f32, tag="ps_big", name="psH")
                            nc.tensor.matmul(out=psH[:], lhsT=xn_t[:],
                                             rhs=wo1_t[:], start=True,
                                             stop=False)
                            nc.tensor.matmul(out=psH[:], lhsT=TT[0][:],
                                             rhs=wo2_0[:], start=False,
                                             stop=False)
                            nc.tensor.matmul(out=psH[:], lhsT=TT[1][:],
                                             rhs=wo2_1[:], start=False,
                                             stop=True)
                            h_sb = sp.tile([P, H], f32, tag="h_sb")
                            nc.scalar.activation(out=h_sb[:], in_=psH[:],
                                                 func=RELU)
                            pw_t = sp.tile([P, GCAP], f32, tag="pw_t")
                            nc.sync.dma_start(out=pw_t[:], in_=poolw_in[t])
                            for s in range(2):
                                nc.tensor.matmul(
                                    out=psG[s][:],
                                    lhsT=pw_t[:, s * P:(s + 1) * P],
                                    rhs=h_sb[:],
                                    start=(t == 0), stop=(t == Kn - 1))
                if not last:
                    nc.gpsimd.collective_compute(
                        "AllGather", mybir.AluOpType.bypass,
                        replica_groups=[list(range(N_CORES))],
                        ins=[ag_in[:].opt()],
                        outs=[U_tab[sw % 2][:].opt()])
                else:
                    for s in range(2):
                        og = sp.tile([P, H], f32, tag="og")
                        nc.vector.tensor_copy(out=og[:], in_=psG[s][:])
                        nc.sync.dma_start(
                            out=out_t[s * P:(s + 1) * P, :], in_=og[:])

    nc.compile()
    return nc


# ======================================================================
# Entry point
# ======================================================================

_last_results = None


def kernel(**inputs):
    from concourse.bass_utils import run_bass_kernel_spmd

    per_core, meta, glo_ghi = _prep(inputs)
    nc = _build(meta)
    in_maps = [{k: v for k, v in pc.items()} for pc in per_core]
    res = run_bass_kernel_spmd(nc, in_maps, core_ids=list(range(N_CORES)))
    global _last_results
    _last_results = res

    G = meta["n_graphs"]
    out = np.zeros((G, H), F32)
    for c in range(N_CORES):
        glo, ghi = glo_ghi[c]
        out[glo:ghi] = res.results[c]["out"][: ghi - glo]
    return out
